# revision 2
# baseline (speedup 1.0000x reference)
"""Trainium2 Bass kernel for strictly-causal RoPE self-attention (no softmax).

  out[b,h] = tril(rope(Q)@rope(Q)^T, -1) @ V    with K = Q.

Sharding: B*H = 8 independent (b,h) slices -> one per NeuronCore (pure data
parallel, no collectives). Per core: T=N=2048.

Per-core pipeline (bf16 matmul / f32 PSUM accumulate), organized to keep the
PE dense from ~5us on (the previous version idled the PE ~60us at the front
waiting for whole-tile RoPE):
  - Inputs arrive t-column-chunked (CW=512): for chunk c, the 8 deinterleaved
    Q tile-pairs + cos/sin table chunks stream in (kk-major), so RoPE and
    stage 1 of superstep c only need chunk c landed, not the full 16MB.
  - RoPE runs column-chunked and is split across DVE (6/8 pairs) and GpSimd
    (2/8 pairs); Activation only does PSUM evictions so it never blocks RoPE.
    PE consumes contraction tiles in production order (DVE halves first).
  - PE order: s1(0) (contraction-outer across its 4 chains so each landed
    RoPE tile is consumed immediately), s1(1), s2(0), s1(2), s2(1), s1(3),
    s2(2), s2(3).  Emitting s1(c+1) before s2(c) hides the pt-evict latency
    at superstep boundaries; V j-blocks are DMA'd between input chunks so
    s2(0) can fill the chunk-1 DMA gap.
  - Strict-causal masks of diagonal 128x128 blocks run on GpSimd, emitted
    between its RoPE chunk shares so they stay timely.
  - Output is written bf16 (half the DMA bytes); host converts to f32.
"""

import os
import sys
import math

for _p in ("/opt/trn_rl_repo", "/root/.axon_site/_ro/trn_rl_repo"):
    if os.path.isdir(_p) and _p not in sys.path:
        sys.path.append(_p)

import numpy as np
import ml_dtypes

B, H, T, N = 2, 4, 2048, 2048
THETA = 2.0 ** 16
NCORES = 8
CW = 512  # superstep width (t-columns) and output n-chunk width

bf16 = ml_dtypes.bfloat16

LAST_RESULT = None  # BassKernelResults of the most recent run (for test.py)


def build_bass(t_len=T, n_dim=N, num_devices=NCORES):
    from concourse import bacc, mybir, tile

    nc = bacc.Bacc("TRN2", target_bir_lowering=False, debug=False,
                   num_devices=num_devices)
    bf = mybir.dt.bfloat16
    f32 = mybir.dt.float32
    mult = mybir.AluOpType.mult

    nh = n_dim // 2
    kh = nh // 128           # te/to pairs (8)
    kk_n = n_dim // 128      # total contraction tiles (16)
    nb = t_len // 128        # t-blocks (16)
    ncks = t_len // CW       # supersteps / column chunks (4)
    sw = CW // 128           # t-blocks per superstep (4)
    nch = n_dim // CW        # output n-chunks (4)
    GP_PAIRS = (6, 7)        # RoPE pairs handled by GpSimd per chunk

    # chunk-major host layouts: rows [1024*c + 128*k : +128] = pair k, chunk c
    qte = nc.declare_dram_parameter("qte", [ncks * nh, CW], bf, isOutput=False)
    qto = nc.declare_dram_parameter("qto", [ncks * nh, CW], bf, isOutput=False)
    cosd = nc.declare_dram_parameter("cosT", [ncks * nh, CW], bf, isOutput=False)
    sind = nc.declare_dram_parameter("sinT", [ncks * nh, CW], bf, isOutput=False)
    vin = nc.declare_dram_parameter("v", [t_len, n_dim], bf, isOutput=False)
    maskd = nc.declare_dram_parameter("mask", [128, 128], bf, isOutput=False)
    outd = nc.declare_dram_parameter("out", [t_len, n_dim], bf, isOutput=True)

    # PE consumes contraction tiles in RoPE production order: DVE pairs'
    # E halves, DVE O halves trail their E by 3 ops, GpSimd pairs last.
    dve_pairs = [k for k in range(kh) if k not in GP_PAIRS]
    kk_order = ([k for k in dve_pairs] + [kh + k for k in dve_pairs]
                + [k for k in GP_PAIRS] + [kh + k for k in GP_PAIRS])

    with tile.TileContext(nc) as tc:
        with (
            tc.tile_pool(name="qrt", bufs=kk_n * ncks) as qrt_pool,
            tc.tile_pool(name="inp", bufs=32) as in_pool,
            tc.tile_pool(name="vres", bufs=nb) as v_pool,
            tc.tile_pool(name="tmpv", bufs=4) as tmpv_pool,
            tc.tile_pool(name="tmpg", bufs=4) as tmpg_pool,
            tc.tile_pool(name="ptile", bufs=30) as p_pool,
            tc.tile_pool(name="osb", bufs=6) as out_pool,
            tc.tile_pool(name="mk", bufs=1) as mk_pool,
            tc.tile_pool(name="psum", bufs=8, space="PSUM") as psum_pool,
        ):
            mask_sb = mk_pool.tile([128, 128], bf)
            nc.sync.dma_start(mask_sb[:], maskd[:])

            # qrt[kk][c] tiles [128, CW]
            qrt = [[None] * ncks for _ in range(kk_n)]

            def dma_chunk_inputs(c):
                tiles = []
                for k in range(kh):
                    r = slice(nh * c + 128 * k, nh * c + 128 * (k + 1))
                    te = in_pool.tile([128, CW], bf, tag="inp")
                    to = in_pool.tile([128, CW], bf, tag="inp")
                    ct = in_pool.tile([128, CW], bf, tag="inp")
                    st = in_pool.tile([128, CW], bf, tag="inp")
                    nc.sync.dma_start(te[:], qte[r, :])
                    nc.sync.dma_start(to[:], qto[r, :])
                    nc.sync.dma_start(ct[:], cosd[r, :])
                    nc.sync.dma_start(st[:], sind[r, :])
                    tiles.append((te, to, ct, st))
                return tiles

            def rope_item(eng, tmp_pool, c, k, tiles):
                te, to, ct, st = tiles[k]
                qe = qrt_pool.tile([128, CW], bf, tag="qrt",
                                   name=f"qe_{k}_{c}")
                qo = qrt_pool.tile([128, CW], bf, tag="qrt",
                                   name=f"qo_{k}_{c}")
                x1 = tmp_pool.tile([128, CW], bf, tag="tmp")
                x2 = tmp_pool.tile([128, CW], bf, tag="tmp")
                eng.tensor_mul(x1[:], to[:], st[:])   # O*S
                eng.tensor_mul(qe[:], te[:], ct[:])   # E*C
                eng.tensor_sub(qe[:], qe[:], x1[:])   # E' = E*C - O*S
                eng.tensor_mul(x2[:], te[:], st[:])   # E*S
                eng.tensor_mul(qo[:], to[:], ct[:])   # O*C
                eng.tensor_add(qo[:], qo[:], x2[:])   # O' = O*C + E*S
                qrt[k][c] = qe
                qrt[kh + k][c] = qo

            def load_v(jlo, jhi):
                for jb in range(jlo, min(jhi, nb)):
                    vt = v_pool.tile([128, n_dim], bf, tag="vt")
                    half = n_dim // 2
                    nc.sync.dma_start(vt[:, 0:half],
                                      vin[128 * jb:128 * (jb + 1), 0:half])
                    nc.sync.dma_start(vt[:, half:n_dim],
                                      vin[128 * jb:128 * (jb + 1), half:n_dim])
                    v_tiles[jb] = vt

            v_tiles = [None] * nb
            pend_mask = []   # diag pt tiles awaiting GpSimd mask emission

            # ---- input DMA + RoPE emission, chunk by chunk ----
            # DMA priority: chunk0, V0-3, chunk1, V4-7, chunk2, V8-11,
            # chunk3, V12-15.  RoPE ops are emitted per chunk right after
            # the chunk's DMAs; engine emission order == (c, k) order.
            chunk_tiles = []
            for c in range(ncks):
                tiles = dma_chunk_inputs(c)
                chunk_tiles.append(tiles)
                load_v(sw * c, sw * (c + 1))

            def emit_rope(c):
                for k in dve_pairs:
                    rope_item(nc.vector, tmpv_pool, c, k, chunk_tiles[c])
                for k in GP_PAIRS:
                    rope_item(nc.gpsimd, tmpg_pool, c, k, chunk_tiles[c])

            def emit_pending_masks():
                for pt in pend_mask:
                    nc.gpsimd.tensor_tensor(pt[:, 0:128], pt[:, 0:128],
                                            mask_sb[:], mult)
                pend_mask.clear()

            def stage1(c, outer):
                t0 = CW * c
                ptiles = {}
                chains = []
                for j in range(sw * c + sw):
                    rj0 = max(128 * j, t0)
                    w = CW * (c + 1) - rj0
                    ps = psum_pool.tile([128, w], f32, tag="psum",
                                        name=f"ps_{c}_{j}")
                    chains.append((j, rj0, w, ps))

                def emit_mm(kk, j, rj0, w, ps, ki):
                    cj, oj = divmod(j, sw)
                    nc.tensor.matmul(
                        ps[:, :],
                        qrt[kk][cj][:, 128 * oj:128 * oj + 128],
                        qrt[kk][c][:, rj0 - t0:rj0 - t0 + w],
                        start=(ki == 0), stop=(ki == kk_n - 1))

                if outer:  # contraction-outer: all chains advance per kk
                    for ki, kk in enumerate(kk_order):
                        for j, rj0, w, ps in chains:
                            emit_mm(kk, j, rj0, w, ps, ki)
                else:
                    for j, rj0, w, ps in chains:
                        for ki, kk in enumerate(kk_order):
                            emit_mm(kk, j, rj0, w, ps, ki)
                for j, rj0, w, ps in chains:
                    pt = p_pool.tile([128, w], bf, tag="pt",
                                     name=f"pt_{c}_{j}")
                    nc.scalar.copy(pt[:, :], ps[:, :])
                    if rj0 == 128 * j:   # diagonal block: strict-causal mask
                        pend_mask.append(pt)
                    ptiles[j] = (pt, rj0)
                return ptiles

            def stage2(c, ptiles):
                for d in range(sw):
                    i = sw * c + d
                    ti = 128 * i
                    for ch in range(nch):
                        ops = psum_pool.tile([128, CW], f32, tag="psum",
                                             name=f"ps2_{i}_{ch}")
                        for j in range(i + 1):
                            pt, rj0 = ptiles[j]
                            off = ti - rj0
                            nc.tensor.matmul(
                                ops[:, :], pt[:, off:off + 128],
                                v_tiles[j][:, CW * ch:CW * (ch + 1)],
                                start=(j == 0), stop=(j == i))
                        osb = out_pool.tile([128, CW], bf, tag="osb",
                                            name=f"osb_{i}_{ch}")
                        nc.scalar.copy(osb[:], ops[:])
                        nc.sync.dma_start(
                            outd[ti:ti + 128, CW * ch:CW * (ch + 1)], osb[:])

            # ---- emission schedule ----
            emit_rope(0)
            pts = {}
            pts[0] = stage1(0, outer=True)
            emit_pending_masks()          # GpSimd: ss0 diag masks
            emit_rope(1)
            pts[1] = stage1(1, outer=False)
            stage2(0, pts[0])
            emit_pending_masks()          # ss1 masks
            emit_rope(2)
            pts[2] = stage1(2, outer=False)
            stage2(1, pts[1])
            emit_pending_masks()          # ss2 masks
            emit_rope(3)
            pts[3] = stage1(3, outer=False)
            stage2(2, pts[2])
            emit_pending_masks()          # ss3 masks
            stage2(3, pts[3])

    nc.compile()
    return nc


def _tables(t_len=T, n_dim=N):
    t = np.arange(n_dim, dtype=np.float32)
    q = np.floor(t / 2.0) * 2.0
    f = (1.0 / THETA ** (q.astype(np.float64) / n_dim)
         / (2.0 * math.pi)).astype(np.float32)
    phases = np.arange(t_len, dtype=np.float32)[:, None] * f[None, :]
    ph = (phases % 1.0) * np.float32(2.0 * math.pi)
    ct = np.ascontiguousarray(np.cos(ph)[:, 0::2].T).astype(bf16)  # [N/2, T]
    st = np.ascontiguousarray(np.sin(ph)[:, 0::2].T).astype(bf16)
    return ct, st


def _chunk_major(x):
    # [nh, T] -> [ncks*nh, CW] with rows [nh*c : nh*(c+1)] = columns chunk c
    nh = x.shape[0]
    ncks = x.shape[1] // CW
    return np.ascontiguousarray(
        x.reshape(nh, ncks, CW).transpose(1, 0, 2).reshape(ncks * nh, CW))


def _mask128():
    s = np.arange(128)[:, None]
    tt = np.arange(128)[None, :]
    return (s < tt).astype(bf16)


_compiled = {}


def _get_nc():
    if "nc" not in _compiled:
        _compiled["nc"] = build_bass()
    return _compiled["nc"]


def kernel(Q, V):
    global LAST_RESULT
    from concourse.bass_utils import run_bass_kernel_spmd

    Q = np.asarray(Q)
    V = np.asarray(V)
    assert Q.shape == (B, H, T, N) and V.shape == (B, H, T, N)

    nc = _get_nc()
    ct, st = _tables()
    ctc, stc = _chunk_major(ct), _chunk_major(st)
    mask = _mask128()

    in_maps = []
    for b in range(B):
        for h in range(H):
            qs = Q[b, h]
            in_maps.append({
                "qte": _chunk_major(
                    np.ascontiguousarray(qs[:, 0::2].T).astype(bf16)),
                "qto": _chunk_major(
                    np.ascontiguousarray(qs[:, 1::2].T).astype(bf16)),
                "cosT": ctc,
                "sinT": stc,
                "v": V[b, h].astype(bf16),
                "mask": mask,
            })

    res = run_bass_kernel_spmd(nc, in_maps, core_ids=list(range(NCORES)))
    LAST_RESULT = res

    out = np.empty((B, H, T, N), dtype=np.float32)
    for b in range(B):
        for h in range(H):
            out[b, h] = res.results[b * H + h]["out"].astype(np.float32)
    return out
